# revision 1
# baseline (speedup 1.0000x reference)
"""Trainium2 Bass kernel for nn_CryptoNet: 3-layer LSTM + per-step BatchNorm + 2-layer head.

Strategy: 8-way data parallel over batch (128 samples/core), zero cross-core
communication (BN uses per-shard batch stats, which the sharding hint allows;
measured rel err vs full-batch stats: 1.8e-3).

Per-core design:
  - batch (128) on SBUF partitions for gate/cell math; recurrent matmuls
    compute gates[128b, 4H] = act.T-stationary @ W.T-moving in float32r
    (full-rate PE) with the bias added via a K=1 ones-row matmul.
  - h is PE-transposed; BN stats via bn_stats/bn_aggr on h.T
    (features-on-partitions); rstd computed with a DVE-only Newton rsqrt
    (keeps the scalar engine on one activation-table set the whole kernel);
    BN apply fused into one scalar-engine activation with per-partition
    scale/bias -> produces z.T directly = next layer's stationary operand.
  - software pipeline across layers: superstep s runs L1@t=s, L2@t=s-1,
    L3@t=s-2, head@t=s-3 so PE/ACT/DVE/DMA overlap across layers.
  - gate order host-permuted to (i,f,o,g): one Sigmoid covers 3H.
  - softmax over 2 classes = sigmoid of logit difference.
"""

import sys
import numpy as np

for p in ("/opt/trn_rl_repo", "/opt/trn_rl_repo/concourse"):
    if p not in sys.path:
        sys.path.insert(0, p)

B, T, I = 1024, 256, 128
T_STEPS = T  # override for small-scale testing
T_RUN = None  # loop steps; defaults to T_STEPS
H1, H2, H3 = 256, 256, 32
NCORES = 8
BL = B // NCORES  # local batch per core = 128
EPS = 1e-5

_CACHE = {}


def _gate_perm(H):
    # torch gate order (i, f, g, o) -> (i, f, o, g)
    idx = np.arange(4 * H)
    i, f, g, o = np.split(idx, 4)
    return np.concatenate([i, f, o, g])


def _build(dt_w, dt_x, run=None):
    import concourse.bass as bass
    import concourse.tile as tile
    import concourse.mybir as mybir
    from concourse import bacc
    from concourse.masks import make_identity

    f32 = mybir.dt.float32
    u32 = mybir.dt.uint32
    f32r = mybir.dt.float32r
    AF = mybir.ActivationFunctionType
    OP = mybir.AluOpType
    r32 = lambda ap: ap.bitcast(f32r)

    nc = bacc.Bacc("TRN2", target_bir_lowering=False, debug=False,
                   num_devices=NCORES)

    with tile.TileContext(nc) as tc:
        dr = lambda name, shape, dt: nc.dram_tensor(
            name, shape, dt, kind="ExternalInput").ap()
        xT = dr("xT", [I, T_STEPS, BL], dt_x)      # host pre-transposed [i, t, b]
        w1t = dr("w1t", [I, 4 * H1], dt_w)         # Wih1.T, gate-reordered
        wh1t = dr("wh1t", [H1, 4 * H1], mybir.dt.bfloat16)
        w2t = dr("w2t", [H2, 4 * H2], mybir.dt.bfloat16)
        wh2t = dr("wh2t", [H2, 4 * H2], mybir.dt.bfloat16)
        w3t = dr("w3t", [H2, 4 * H3], mybir.dt.bfloat16)
        wh3ta = dr("wh3ta", [H3 + 1, 4 * H3], mybir.dt.bfloat16)  # [Whh3.T ; b3]
        b1r = dr("b1r", [1, 4 * H1], dt_w)
        b2r = dr("b2r", [1, 4 * H2], dt_w)
        gball = dr("gball", [128, 10], f32)  # gamma cols 0:5, beta cols 5:10
        wlt = dr("wlt", [H3, 2], mybir.dt.bfloat16)       # Wl.T
        blp = dr("blp", [2, 1], f32)         # bl as per-partition bias
        wd = dr("wd", [2, 1], mybir.dt.bfloat16)          # Wl2[0]-Wl2[1] as column
        headc = dr("headc", [128, 1], f32)   # bl2[0]-bl2[1] replicated
        y = nc.dram_tensor("y", [BL, 2 * T_STEPS], f32,
                           kind="ExternalOutput").ap()

        with (
            tc.tile_pool(name="const", bufs=1) as const,
            tc.tile_pool(name="state", bufs=1) as state,
            tc.tile_pool(name="xin", bufs=3) as xin,
            tc.tile_pool(name="work", bufs=3) as work,
            tc.tile_pool(name="zt", bufs=3) as ztp,
            tc.tile_pool(name="g1p", bufs=1, space="PSUM") as g1p,
            tc.tile_pool(name="g2p", bufs=1, space="PSUM") as g2p,
            tc.tile_pool(name="smp", bufs=1, space="PSUM") as smp,
            tc.tile_pool(name="tp1", bufs=1, space="PSUM") as tp1,
            tc.tile_pool(name="tp2", bufs=1, space="PSUM") as tp2,
            tc.tile_pool(name="hdp", bufs=1, space="PSUM") as hdp,
        ):
            # ---------------- constants ----------------
            ident = const.tile([128, 128], f32)
            make_identity(nc, ident)
            bf16 = mybir.dt.bfloat16
            ident_b = const.tile([128, 128], bf16)
            make_identity(nc, ident_b)
            ones_row = const.tile([1, 128], dt_w)
            nc.vector.memset(ones_row.bitcast(f32), 1.0)
            magic_t = const.tile([128, 5], u32)
            nc.vector.memset(magic_t, 0x5F3759DF)

            def load(name, shape, dt, src):
                t = const.tile(shape, dt, tag=name)
                nc.sync.dma_start(t[:], src)
                return t

            w1t_s = load("w1t", [128, 4 * H1], dt_w, w1t[:])
            wh1t_s = load("wh1t", [128, 2, 4 * H1], bf16,
                          wh1t.rearrange("(k p) n -> p k n", p=128))
            w2t_s = load("w2t", [128, 2, 4 * H2], bf16,
                         w2t.rearrange("(k p) n -> p k n", p=128))
            wh2t_s = load("wh2t", [128, 2, 4 * H2], bf16,
                          wh2t.rearrange("(k p) n -> p k n", p=128))
            w3t_s = load("w3t", [128, 2, 4 * H3], bf16,
                         w3t.rearrange("(k p) n -> p k n", p=128))
            wh3ta_s = load("wh3ta", [H3 + 1, 4 * H3], bf16, wh3ta[:])
            b1r_s = load("b1r", [1, 4 * H1], dt_w, b1r[:])
            b2r_s = load("b2r", [1, 4 * H2], dt_w, b2r[:])
            gball_s = load("gball", [128, 10], f32, gball[:])
            wlt_s = load("wlt", [H3, 2], bf16, wlt[:])
            blp_s = load("blp", [2, 1], f32, blp[:])
            wd_s = load("wd", [2, 1], bf16, wd[:])
            headc_s = load("headc", [128, 1], f32, headc[:])
            nheadc_s = const.tile([128, 1], f32)
            nc.vector.tensor_scalar_mul(nheadc_s, headc_s, -1.0)

            # ---------------- persistent state ----------------
            c1 = state.tile([128, H1], f32)
            c2 = state.tile([128, H2], f32)
            c3 = state.tile([128, H3], f32)
            h1T = state.tile([128, 2, 128], bf16)   # feat-part, batch-free
            h2T = state.tile([128, 2, 128], bf16)
            h3Ta = state.tile([H3 + 1, 128], f32)  # last row = ones (bias)
            out_sb = state.tile([128, 2 * T_STEPS], f32)
            for tens in (c1, c2, c3):
                nc.vector.memset(tens, 0.0)
            for tens in (h1T, h2T):
                nc.vector.memset(tens, 0.0)
            nc.vector.memset(h3Ta[0:H3, :], 0.0)
            nc.vector.memset(h3Ta[H3:H3 + 1, :], 1.0)

            XCH = 8  # x chunk length (steps per DMA)

            def cell_math(gates, H, c, tag, split=False, dth=None):
                """sigmoid/tanh + cell update; returns h [128, H] (BF).

                split=True issues sigmoid(i,f) on the first PSUM bank so it
                can start before the second bank's matmuls finish."""
                dth = dth or f32
                sig = work.tile([128, 3 * H], dth, tag=f"sig{tag}")
                tg = work.tile([128, H], dth, tag=f"tg{tag}")
                cn = work.tile([128, H], f32, tag=f"cn{tag}")
                tm = work.tile([128, H], dth, tag=f"tm{tag}")
                h = work.tile([128, H], dth, tag=f"h{tag}")
                if split:
                    nc.scalar.activation(sig[:, 0:2 * H], gates[:, 0:2 * H],
                                         AF.Sigmoid)
                else:
                    nc.scalar.activation(sig, gates[:, 0:3 * H], AF.Sigmoid)
                nc.vector.tensor_mul(cn, sig[:, H:2 * H], c)     # f*c first
                nc.scalar.activation(tg, gates[:, 3 * H:4 * H], AF.Tanh)
                if split:
                    nc.scalar.activation(sig[:, 2 * H:3 * H],
                                         gates[:, 2 * H:3 * H], AF.Sigmoid)
                nc.vector.tensor_mul(tm, sig[:, 0:H], tg)        # i*g~
                nc.vector.tensor_add(c, cn, tm)
                nc.scalar.activation(tg, c, AF.Tanh)             # tanh(c)
                nc.vector.tensor_mul(h, sig[:, 2 * H:3 * H], tg)
                return h

            R = run if run is not None else T_STEPS
            NS = R + 6
            mvq = {}

            def get_mv(i):
                if i not in mvq:
                    mvq[i] = work.tile([128, 5, 2], f32, tag="mv", bufs=4,
                                       name="mvt")
                return mvq[i]

            h3q = {}
            h3init = work.tile([H3 + 1, 128], bf16, tag="h3a", bufs=4,
                               name="h3init")
            nc.vector.memset(h3init[0:H3, :], 0.0)
            nc.vector.memset(h3init[H3:H3 + 1, :], 1.0)
            h3q[2] = h3init
            z1T_prev = z2T_prev = z3T_prev = None
            h2T_ps_prev = None

            for s in range(NS):
                st6 = work.tile([128, 5, 6], f32, tag="st6")
                h1T_ps = h2T_ps = None

                # ---------- L1 @ t=s ----------
                if s < R:
                    ti = s % XCH
                    if ti == 0:
                        xT_sb = xin.tile([128, XCH, 128], dt_x, tag="x")
                        nc.sync.dma_start(xT_sb, xT[:, s:s + XCH, :])
                    g1 = g1p.tile([128, 4 * H1], f32, tag="g1")
                    for nj in range(2):
                        nn_ = slice(512 * nj, 512 * (nj + 1))
                        nc.tensor.matmul(g1[:, nn_], ones_row, b1r_s[:, nn_],
                                         start=True, stop=False)
                        nc.tensor.matmul(g1[:, nn_], xT_sb[:, ti, :],
                                         w1t_s[:, nn_], start=False, stop=False)
                        for k in range(2):
                            nc.tensor.matmul(g1[:, nn_], h1T[:, k, :],
                                             wh1t_s[:, k, nn_],
                                             start=False, stop=(k == 1))
                    h1 = cell_math(g1, H1, c1, "a", split=True, dth=bf16)
                    h1T_ps = tp1.tile([128, 2, 128], bf16, tag="tp1")
                    for j in range(2):
                        nc.tensor.transpose(h1T_ps[:, j, :],
                                            h1[:, j * 128:(j + 1) * 128],
                                            ident_b)
                    nc.vector.tensor_copy(h1T[:, :, :], h1T_ps[:, :, :])
                    mvc = get_mv(s)
                    for j in range(2):
                        nc.vector.bn_stats(st6[:, j, :], h1T_ps[:, j, :])
                        nc.vector.bn_aggr(mvc[:, j, :], st6[:, j, :])

                # ---------- L2 @ t=s-1 ----------
                if 1 <= s <= R:
                    z1T = z1T_prev
                    g2 = g2p.tile([128, 4 * H2], f32, tag="g2")
                    for nj in range(2):
                        nn_ = slice(512 * nj, 512 * (nj + 1))
                        nc.tensor.matmul(g2[:, nn_], ones_row, b2r_s[:, nn_],
                                         start=True, stop=False)
                        for k in range(2):
                            nc.tensor.matmul(g2[:, nn_], z1T[:, k, :],
                                             w2t_s[:, k, nn_],
                                             start=False, stop=False)
                        for k in range(2):
                            nc.tensor.matmul(g2[:, nn_], h2T[:, k, :],
                                             wh2t_s[:, k, nn_],
                                             start=False, stop=(k == 1))
                    h2 = cell_math(g2, H2, c2, "b", split=True, dth=bf16)
                    h2T_ps = tp2.tile([128, 2, 128], bf16, tag="tp2")
                    for j in range(2):
                        nc.tensor.transpose(h2T_ps[:, j, :],
                                            h2[:, j * 128:(j + 1) * 128],
                                            ident_b)
                    nc.vector.tensor_copy(h2T[:, :, :], h2T_ps[:, :, :])
                    mvn = get_mv(s + 1)
                    for j in range(2):
                        nc.vector.bn_stats(st6[:, 2 + j, :], h2T_ps[:, j, :])
                        nc.vector.bn_aggr(mvn[:, 2 + j, :], st6[:, 2 + j, :])

                # ---------- L3 @ t=s-3 ----------
                if 3 <= s <= R + 2:
                    z2T = z2T_prev
                    g3 = smp.tile([128, 4 * H3], f32, tag="sm")
                    nc.tensor.matmul(g3, z2T[:, 0, :], w3t_s[:, 0, :],
                                     start=True, stop=False)
                    nc.tensor.matmul(g3, z2T[:, 1, :], w3t_s[:, 1, :],
                                     start=False, stop=False)
                    nc.tensor.matmul(g3, h3q[s - 1], wh3ta_s,
                                     start=False, stop=True)
                    h3 = cell_math(g3, H3, c3, "c", dth=bf16)
                    h3T_ps = smp.tile([H3, 128], bf16, tag="sm")
                    nc.tensor.transpose(h3T_ps, h3[:, 0:H3], ident_b)
                    h3aug = work.tile([H3 + 1, 128], bf16, tag="h3a", bufs=4)
                    nc.vector.tensor_copy(h3aug[0:H3, :], h3T_ps)
                    nc.vector.memset(h3aug[H3:H3 + 1, :], 1.0)
                    mvn2 = get_mv(s + 2)
                    nc.vector.bn_stats(st6[0:H3, 4, :], h3aug[0:H3, :])
                    nc.vector.bn_aggr(mvn2[0:H3, 4, :], st6[0:H3, 4, :])
                    h3q[s] = h3aug

                # ---------- head @ t=s-6 ----------
                if 6 <= s <= R + 5:
                    t_out = s - 6
                    z3T = z3T_prev
                    o1t = smp.tile([2, 128], f32, tag="sm")
                    nc.tensor.matmul(o1t, wlt_s, z3T, start=True, stop=True)
                    relu1 = work.tile([2, 128], bf16, tag="rl")
                    nc.scalar.activation(relu1, o1t, AF.Relu, bias=blp_s)
                    dcol = smp.tile([128, 1], f32, tag="sm")
                    nc.tensor.matmul(dcol, relu1, wd_s, start=True, stop=True)
                    nc.scalar.activation(out_sb[:, 2 * t_out:2 * t_out + 1],
                                         dcol, AF.Sigmoid, bias=headc_s,
                                         scale=1.0)
                    nc.vector.tensor_scalar(
                        out=out_sb[:, 2 * t_out + 1:2 * t_out + 2],
                        in0=out_sb[:, 2 * t_out:2 * t_out + 1],
                        scalar1=-1.0, scalar2=1.0, op0=OP.mult, op1=OP.add)

                if s <= R + 4:
                    # ---- fused Newton rsqrt + coefs (slack tail) ----
                    mv = get_mv(s)
                    cs = slice(0, 5)
                    ve = work.tile([128, 5], f32, tag="ve")
                    t2 = work.tile([128, 5], f32, tag="t2")
                    u2 = work.tile([128, 5], f32, tag="u2")
                    y1 = work.tile([128, 5], f32, tag="y1")
                    s_ = work.tile([128, 5], f32, tag="s_")
                    tt = work.tile([128, 5], f32, tag="tt")
                    vecs = ve[:, cs]
                    nc.vector.tensor_scalar_add(vecs, mv[:, cs, 1], EPS)
                    nc.vector.tensor_scalar(
                        out=t2[:, cs].bitcast(u32), in0=vecs.bitcast(u32),
                        scalar1=1, scalar2=None, op0=OP.arith_shift_right)
                    nc.gpsimd.tensor_sub(y1[:, cs].bitcast(u32),
                                         magic_t[:, cs], t2[:, cs].bitcast(u32))
                    nc.gpsimd.tensor_mul(u2[:, cs], y1[:, cs], y1[:, cs])
                    nc.gpsimd.tensor_mul(t2[:, cs], vecs, u2[:, cs])
                    nc.vector.tensor_scalar(out=u2[:, cs], in0=t2[:, cs],
                                            scalar1=-0.5, scalar2=1.5,
                                            op0=OP.mult, op1=OP.add)
                    nc.gpsimd.tensor_mul(t2[:, cs], y1[:, cs], u2[:, cs])
                    nc.gpsimd.tensor_mul(u2[:, cs], t2[:, cs], t2[:, cs])
                    nc.gpsimd.tensor_mul(y1[:, cs], vecs, u2[:, cs])
                    nc.vector.tensor_scalar(out=u2[:, cs], in0=y1[:, cs],
                                            scalar1=-0.5, scalar2=1.5,
                                            op0=OP.mult, op1=OP.add)
                    nc.gpsimd.tensor_mul(y1[:, cs], t2[:, cs], u2[:, cs])
                    nc.gpsimd.tensor_mul(s_[:, cs], y1[:, cs], gball_s[:, cs])
                    nc.gpsimd.tensor_mul(u2[:, cs], mv[:, cs, 0], s_[:, cs])
                    nc.gpsimd.tensor_sub(tt[:, cs], gball_s[:, 5:10], u2[:, cs])

                    # ---- BN applies on the h-states saved last superstep ----
                if s < R:
                    z1T = ztp.tile([128, 2, 128], bf16, tag="z1")
                    for j in range(2):
                        nc.vector.tensor_scalar(
                            out=z1T[:, j, :], in0=h1T[:, j, :],
                            scalar1=s_[:, j:j + 1], scalar2=tt[:, j:j + 1],
                            op0=OP.mult, op1=OP.add)
                    z1T_prev = z1T
                if 2 <= s <= R + 1:
                    z2T = ztp.tile([128, 2, 128], bf16, tag="z2")
                    for j in range(2):
                        nc.vector.tensor_scalar(
                            out=z2T[:, j, :], in0=h2T_ps_prev[:, j, :],
                            scalar1=s_[:, 2 + j:3 + j],
                            scalar2=tt[:, 2 + j:3 + j],
                            op0=OP.mult, op1=OP.add)
                    z2T_prev = z2T
                if 5 <= s <= R + 4:
                    z3T = ztp.tile([H3, 128], bf16, tag="z3")
                    nc.vector.tensor_scalar(
                        out=z3T, in0=h3q[s - 2][0:H3, :],
                        scalar1=s_[0:H3, 4:5], scalar2=tt[0:H3, 4:5],
                        op0=OP.mult, op1=OP.add)
                    z3T_prev = z3T
                h2T_ps_prev = h2T_ps
                if s - 3 in h3q:
                    del h3q[s - 3]


            nc.sync.dma_start(y, out_sb)

    nc.compile()
    return nc


def _prep_host(inputs, np_w, np_x):
    gp1 = _gate_perm(H1)
    gp2 = _gate_perm(H2)
    gp3 = _gate_perm(H3)
    f = lambda a: np.ascontiguousarray(a, dtype=np.float32)

    import ml_dtypes
    bf = ml_dtypes.bfloat16
    w1t = f(inputs["Wih1"][gp1].T).astype(np_w)
    wh1t = f(inputs["Whh1"][gp1].T).astype(bf)
    w2t = f(inputs["Wih2"][gp2].T).astype(bf)
    wh2t = f(inputs["Whh2"][gp2].T).astype(bf)
    w3t = f(inputs["Wih3"][gp3].T).astype(bf)
    wh3t = f(inputs["Whh3"][gp3].T).astype(bf)
    b1 = f(inputs["bih1"] + inputs["bhh1"])[gp1][None, :]
    b2 = f(inputs["bih2"] + inputs["bhh2"])[gp2][None, :]
    b3 = f(inputs["bih3"] + inputs["bhh3"])[gp3][None, :]
    wh3ta = np.concatenate([wh3t, b3.astype(bf)], axis=0)

    def cols128(v):  # [256] -> [128, 2]
        return np.ascontiguousarray(v.reshape(2, 128).T, dtype=np.float32)

    gball = np.zeros((128, 10), np.float32)
    gball[:, 0:2] = cols128(f(inputs["g1"]))
    gball[:, 2:4] = cols128(f(inputs["g2"]))
    gball[0:H3, 4] = f(inputs["g3"])
    gball[:, 5:7] = cols128(f(inputs["b1"]))
    gball[:, 7:9] = cols128(f(inputs["b2"]))
    gball[0:H3, 9] = f(inputs["b3"])

    wlt = f(inputs["Wl"].T).astype(bf)
    blp = f(inputs["bl"])[:, None]
    wd = f(inputs["Wl2"][0] - inputs["Wl2"][1])[:, None].astype(bf)
    dc = float(inputs["bl2"][0] - inputs["bl2"][1])
    headc = np.full((128, 1), dc, np.float32)

    shared = dict(w1t=w1t, wh1t=wh1t, w2t=w2t, wh2t=wh2t, w3t=w3t,
                  wh3ta=wh3ta, b1r=b1, b2r=b2, gball=gball,
                  wlt=wlt, blp=blp, wd=wd, headc=headc)

    x = np.asarray(inputs["x"], dtype=np.float32)
    in_maps = []
    for c in range(NCORES):
        xc = x[c * BL:(c + 1) * BL]
        xTc = np.ascontiguousarray(
            xc[:, :T_STEPS, :].transpose(2, 1, 0)).astype(np_x)
        m = dict(shared)
        m["xT"] = xTc
        in_maps.append(m)
    return in_maps


def kernel(**inputs):
    import concourse.mybir as mybir
    from concourse import bass_utils

    dt_w = mybir.dt.float32r
    dt_x = mybir.dt.float32r
    np_w = np.float32
    np_x = np.float32

    key = ("v3", str(dt_w), str(dt_x), T_STEPS, T_RUN)
    if key not in _CACHE:
        _CACHE[key] = _build(dt_w, dt_x, run=T_RUN)
    nc = _CACHE[key]

    in_maps = _prep_host(inputs, np_w, np_x)
    res = bass_utils.run_bass_kernel_spmd(nc, in_maps,
                                          core_ids=list(range(NCORES)))
    out = np.empty((B, T_STEPS, 2), np.float32)
    for c in range(NCORES):
        out[c * BL:(c + 1) * BL] = res.results[c]["y"].reshape(BL, T_STEPS, 2)
    return out



# revision 2
# speedup vs baseline: 1.0831x; 1.0831x over previous
"""Trainium2 Bass kernel v2 for nn_CryptoNet: 3-layer LSTM + per-step BatchNorm
+ 2-layer softmax head. 8-way data parallel over batch (128/core), per-shard
BN stats.

Design vs baseline (1.78ms):
  - Gates computed TRANSPOSED [4H on partitions, batch on free] so the
    recurrent h.T is produced in matmul-ready layout: no PE transposes, no
    PSUM->SBUF h copies.
  - All large matmuls are fp8e4 DoubleRow (0.5 cyc/row): stationary weights
    [K,2,M], moving activations [K,2,N]; K-dim is free so biases ride
    augmented K rows (hi+lo fp8 split for precision).
  - tanh(g) folded into the sigmoid instruction via sigma(2g) (host
    premultiplies g-gate rows by 2); cell math uses fused
    scalar_tensor_tensor ops: ONE activation instr per layer-group + one
    tanh(c) instr -> 4 ACT instrs/step total.
  - BN mean/sumsq come free from accum_out on the h-mul / square STT ops;
    rstd via 1-iter Newton on magic-seed (negated-gamma trick absorbs the
    seed sign), all on DVE to keep the coef chain latency low.
  - L3 (H3=32) runs batch-on-partitions (orientation A) with a single PE
    transpose; its sigma rides the L2 sigma instr, its tanh rides tanh(c2).
  - head: softmax(2) == (sigma(d), sigma(-d)) with d = logit diff; both
    computed by appending 2 PSUM cols to the L2/L3 sigma instr.
"""

import sys
import numpy as np

for p in ("/opt/trn_rl_repo", "/opt/trn_rl_repo/concourse"):
    if p not in sys.path:
        sys.path.insert(0, p)

B, T, I = 1024, 256, 128
T_STEPS = T
H1, H2, H3 = 256, 256, 32
NCORES = 8
BL = B // NCORES
EPS = 1e-5

_CACHE = {}


def _build(run=None):
    import concourse.bass as bass
    import concourse.tile as tile
    import concourse.mybir as mybir
    from concourse import bacc
    from concourse.masks import make_identity

    f32 = mybir.dt.float32
    u32 = mybir.dt.uint32
    bf16 = mybir.dt.bfloat16
    f16 = mybir.dt.float16
    fp8 = mybir.dt.float8e4
    AF = mybir.ActivationFunctionType
    OP = mybir.AluOpType
    DR = mybir.MatmulPerfMode.DoubleRow

    R = run if run is not None else T_STEPS
    NS = R + 7
    XCH = 8

    nc = bacc.Bacc("TRN2", target_bir_lowering=False, debug=False,
                   num_devices=NCORES)

    with tile.TileContext(nc) as tc:
        dram = lambda name, shape, dt: nc.dram_tensor(
            name, shape, dt, kind="ExternalInput").ap()
        xdr = dram("xdr", [65, 2, T_STEPS, 128], fp8)
        w1x = dram("w1x", [65, 2, 8, 128], fp8)
        wh1 = dram("wh1", [128, 2, 8, 128], fp8)
        w2z = dram("w2z", [128, 2, 8, 128], fp8)
        wh2 = dram("wh2", [128, 2, 8, 128], fp8)
        b2l = dram("b2l", [1, 2, 8, 128], fp8)
        w3t = dram("w3t", [128, 2, 128], fp8)
        wh3a = dram("wh3a", [33, 128], bf16)
        wlz = dram("wlz", [33, 2], bf16)
        wd2 = dram("wd2", [3, 2], bf16)
        gam = dram("gam", [128, 5], f32)
        bet = dram("bet", [128, 5], f32)
        onesd = dram("onesd", [1, 2, 128], fp8)
        y = nc.dram_tensor("y", [BL, 2 * T_STEPS], f32,
                           kind="ExternalOutput").ap()

        with (
            tc.tile_pool(name="const", bufs=1) as const,
            tc.tile_pool(name="state", bufs=1) as state,
            tc.tile_pool(name="xin", bufs=3) as xin,
            tc.tile_pool(name="work", bufs=3) as work,
            tc.tile_pool(name="g1p", bufs=1, space="PSUM") as g1p,
            tc.tile_pool(name="g23p", bufs=1, space="PSUM") as g23p,
            tc.tile_pool(name="smp", bufs=1, space="PSUM") as smp,
        ):
            # ---------------- constants ----------------
            ident_f = const.tile([128, 128], f16)
            make_identity(nc, ident_f)
            ones32 = const.tile([32, 128], bf16)
            nc.vector.memset(ones32, 1.0)
            magic = const.tile([128, 5], u32)
            nc.vector.memset(magic, 0x5F3759DF)

            def load(name, shape, dt, src):
                t = const.tile(shape, dt, tag=name)
                nc.sync.dma_start(t[:], src)
                return t

            w1x_s = load("w1x", [65, 2, 8, 128], fp8, w1x[:])
            wh1_s = load("wh1", [128, 2, 8, 128], fp8, wh1[:])
            w2z_s = load("w2z", [128, 2, 8, 128], fp8, w2z[:])
            wh2_s = load("wh2", [128, 2, 8, 128], fp8, wh2[:])
            b2l_s = load("b2l", [1, 2, 8, 128], fp8, b2l[:])
            w3t_s = load("w3t", [128, 2, 128], fp8, w3t[:])
            wh3a_s = load("wh3a", [33, 128], bf16, wh3a[:])
            wlz_s = load("wlz", [33, 2], bf16, wlz[:])
            wd2_s = load("wd2", [3, 2], bf16, wd2[:])
            gam_s = load("gam", [128, 5], f32, gam[:])
            bet_s = load("bet", [128, 5], f32, bet[:])
            onesdr = load("onesd", [1, 2, 128], fp8, onesd[:])

            # ---------------- persistent state ----------------
            c1 = state.tile([128, 256], bf16)
            c23 = state.tile([128, 288], bf16)   # c2 cols 0:256, c3 256:288
            h1T = state.tile([128, 2, 128], fp8)
            h2T = state.tile([128, 2, 128], fp8)
            z1q = [state.tile([128, 2, 128], fp8, tag=f"z1q{k}",
                              name=f"z1q{k}") for k in range(2)]
            z2q = [state.tile([128, 2, 128], fp8, tag=f"z2q{k}",
                              name=f"z2q{k}") for k in range(3)]
            h3q = [state.tile([33, 128], bf16, tag=f"h3q{k}", name=f"h3q{k}")
                   for k in range(3)]            # row 32 = ones
            z3q = [state.tile([33, 128], bf16, tag=f"z3q{k}", name=f"z3q{k}")
                   for k in range(2)]            # row 32 = ones
            relu1a = state.tile([3, 128], bf16)  # row 2 = ones
            out_sb = state.tile([128, 2 * T_STEPS], f32)
            for t_ in (c1, c23):
                nc.vector.memset(t_, 0.0)
            for t_ in [h1T, h2T] + z1q + z2q:
                nc.vector.memset(t_.bitcast(u32), 0)
            for t_ in h3q + z3q:
                nc.vector.memset(t_[0:32, :], 0.0)
                nc.vector.memset(t_[32:33, :], 1.0)
            nc.vector.memset(relu1a, 1.0)  # rows 0:2 overwritten by relu

            # rotating BN stat tiles: mean/sumsq written @s, chain reads @s+1
            mvq = []
            for k in range(3):
                m = state.tile([128, 5], f32, tag=f"mvS{k}")
                s2 = state.tile([128, 5], f32, tag=f"mvQ{k}")
                nc.vector.memset(m, 0.0)
                nc.vector.memset(s2, 1.0)
                mvq.append((m, s2))

            def get_mv(i):
                return mvq[i % 3]

            STT = nc.vector.scalar_tensor_tensor

            for s in range(NS):
                # ---------------- x stream ----------------
                if s < R:
                    ti = s % XCH
                    if ti == 0:
                        xT_sb = xin.tile([65, 2, XCH, 128], fp8, tag="x")
                        nc.sync.dma_start(xT_sb, xdr[:, :, s:s + XCH, :])

                # ---- head front half for t=s-7 (all inputs from s-1) ----
                if 7 <= s <= R + 6:
                    o1t = smp.tile([2, 128], f32, tag="o1t")
                    nc.tensor.matmul(o1t, wlz_s, z3q[(s - 1) % 2],
                                     start=True, stop=True)
                    nc.vector.tensor_scalar_max(relu1a[0:2, :], o1t, 0.0)

                # ===== BN coef chain (stats from s-1): all-DVE =====
                if 1 <= s <= R + 5:
                    mvP, mvR = get_mv(s - 1)
                    mu = work.tile([128, 5], f32, tag="mu")
                    v = work.tile([128, 5], f32, tag="v")
                    q = work.tile([128, 5], f32, tag="q")
                    yns = work.tile([128, 5], f32, tag="yns")
                    y2 = work.tile([128, 5], f32, tag="y2")
                    w_ = work.tile([128, 5], f32, tag="w_")
                    sco = work.tile([128, 5], f32, tag="sco")
                    tco = work.tile([128, 5], f32, tag="tco")
                    nc.vector.tensor_scalar_mul(mu, mvP, 1.0 / 128.0)
                    nc.vector.tensor_scalar(out=q, in0=mvR, scalar1=1.0 / 128.0,
                                            scalar2=EPS, op0=OP.mult, op1=OP.add)
                    nc.gpsimd.tensor_tensor(y2, mu, mu, OP.mult)
                    nc.gpsimd.tensor_tensor(v, q, y2, OP.subtract)
                    nc.vector.tensor_scalar(
                        out=w_.bitcast(u32), in0=v.bitcast(u32), scalar1=1,
                        scalar2=None, op0=OP.arith_shift_right)
                    nc.gpsimd.tensor_tensor(yns.bitcast(u32), magic,
                                            w_.bitcast(u32), OP.subtract)
                    nc.gpsimd.tensor_tensor(y2, yns, yns, OP.mult)
                    nc.gpsimd.tensor_tensor(w_, v, y2, OP.mult)
                    nc.vector.tensor_scalar(out=w_, in0=w_, scalar1=-0.5,
                                            scalar2=1.5, op0=OP.mult, op1=OP.add)
                    nc.gpsimd.tensor_tensor(yns, yns, w_, OP.mult)
                    nc.gpsimd.tensor_tensor(sco, yns, gam_s, OP.mult)
                    nc.gpsimd.tensor_tensor(q, mu, sco, OP.mult)
                    nc.gpsimd.tensor_tensor(tco, bet_s, q, OP.subtract)

                    if s <= R:           # z1 for t=s-1
                        z1c = z1q[(s - 1) % 2]
                        for j in range(2):
                            nc.vector.tensor_scalar(
                                out=z1c[:, j, :], in0=h1T[:, j, :],
                                scalar1=sco[:, j:j + 1], scalar2=tco[:, j:j + 1],
                                op0=OP.mult, op1=OP.add)
                    if 3 <= s <= R + 2:  # z2 for t=s-3
                        z2c = z2q[s % 3]
                        for j in range(2):
                            nc.vector.tensor_scalar(
                                out=z2c[:, j, :], in0=h2T[:, j, :],
                                scalar1=sco[:, 2 + j:3 + j],
                                scalar2=tco[:, 2 + j:3 + j],
                                op0=OP.mult, op1=OP.add)
                    if 6 <= s <= R + 5:  # z3 for t=s-6
                        nc.vector.tensor_scalar(
                            out=z3q[s % 2][0:32, :],
                            in0=h3q[(s - 2) % 3][0:32, :],
                            scalar1=sco[0:32, 4:5], scalar2=tco[0:32, 4:5],
                            op0=OP.mult, op1=OP.add)

                mvS, mvQ = get_mv(s)

                # ================= PE matmuls =================
                if s < R:
                    g1 = g1p.tile([128, 1024], f32, tag="g1")
                    for c in range(8):
                        cs = slice(128 * c, 128 * (c + 1))
                        nc.tensor.matmul(g1[:, cs], w1x_s[:, :, c, :],
                                         xT_sb[:, :, ti, :],
                                         start=True, stop=False, perf_mode=DR)
                        nc.tensor.matmul(g1[:, cs], wh1_s[:, :, c, :], h1T,
                                         start=False, stop=True, perf_mode=DR)
                g23 = g23p.tile([128, 1154], f32, tag="g23")
                if 2 <= s <= R + 1:      # L2 for t=s-2
                    z1v = z1q[s % 2]
                    for c in range(8):
                        cs = slice(128 * c, 128 * (c + 1))
                        nc.tensor.matmul(g23[:, cs], b2l_s[:, :, c, :], onesdr,
                                         start=True, stop=False, perf_mode=DR)
                        nc.tensor.matmul(g23[:, cs], w2z_s[:, :, c, :], z1v,
                                         start=False, stop=False, perf_mode=DR)
                        nc.tensor.matmul(g23[:, cs], wh2_s[:, :, c, :], h2T,
                                         start=False, stop=True, perf_mode=DR)
                if 4 <= s <= R + 3:      # L3 for t=s-4
                    nc.tensor.matmul(g23[:, 1024:1152], z2q[(s - 1) % 3], w3t_s,
                                     start=True, stop=False, perf_mode=DR)
                    nc.tensor.matmul(g23[:, 1024:1152], h3q[(s - 1) % 3],
                                     wh3a_s, start=False, stop=True)
                if 7 <= s <= R + 6:      # head logits for t=s-7
                    nc.tensor.matmul(g23[:, 1152:1154], relu1a, wd2_s,
                                     start=True, stop=True)

                # ================= ACT + cell math =================
                W23 = 1154 if 7 <= s else (1152 if 4 <= s else 1024)
                if 2 <= s <= R + 6:
                    s23b = work.tile([128, 1154], f16, tag="s23b")
                    nc.scalar.activation(s23b[:, 0:W23], g23[:, 0:W23],
                                         AF.Sigmoid)
                if s < R:
                    s1b = work.tile([128, 1024], f16, tag="s1b")
                    nc.scalar.activation(s1b, g1, AF.Sigmoid)
                    u1 = work.tile([128, 256], f16, tag="u1")
                    mfc1 = work.tile([128, 256], bf16, tag="mfc1")
                    STT(u1, s1b[:, 768:1024], 0.5, s1b[:, 0:256],
                        OP.subtract, OP.mult)
                    nc.gpsimd.tensor_tensor(mfc1, s1b[:, 256:512], c1, OP.mult)
                    STT(c1, u1, 2.0, mfc1, OP.mult, OP.add)
                    tc1 = work.tile([128, 256], f16, tag="tc1")
                    nc.scalar.activation(tc1, c1, AF.Tanh)

                if 2 <= s <= R + 1:
                    u2 = work.tile([128, 256], f16, tag="u2")
                    mfc2 = work.tile([128, 256], bf16, tag="mfc2")
                    STT(u2, s23b[:, 768:1024], 0.5, s23b[:, 0:256],
                        OP.subtract, OP.mult)
                    nc.gpsimd.tensor_tensor(mfc2, s23b[:, 256:512],
                                            c23[:, 0:256], OP.mult)
                    STT(c23[:, 0:256], u2, 2.0, mfc2, OP.mult, OP.add)
                    tc2 = work.tile([128, 256], f16, tag="tc2")
                    nc.scalar.activation(tc2, c23[:, 0:256], AF.Tanh)
                if 4 <= s <= R + 3:
                    u3 = work.tile([128, 32], f16, tag="u3")
                    mfc3 = work.tile([128, 32], bf16, tag="mfc3")
                    STT(u3, s23b[:, 1120:1152], 0.5, s23b[:, 1024:1056],
                        OP.subtract, OP.mult)
                    nc.gpsimd.tensor_tensor(mfc3, s23b[:, 1056:1088],
                                            c23[:, 256:288], OP.mult)
                    STT(c23[:, 256:288], u3, 2.0, mfc3, OP.mult, OP.add)
                    tc3 = work.tile([128, 32], f16, tag="tc3")
                    nc.scalar.activation(tc3, c23[:, 256:288], AF.Tanh)

                if s < R:
                    sqd = work.tile([128, 128], bf16, tag="sqd")
                    for j in range(2):
                        js = slice(128 * j, 128 * (j + 1))
                        STT(h1T[:, j, :], s1b[:, 512 + 128 * j:640 + 128 * j],
                            1.0, tc1[:, js], OP.mult, OP.mult,
                            accum_out=mvS[:, j:j + 1])
                        STT(sqd, h1T[:, j, :], 1.0, h1T[:, j, :],
                            OP.mult, OP.mult, accum_out=mvQ[:, j:j + 1])
                if 2 <= s <= R + 1:
                    sqd2 = work.tile([128, 128], bf16, tag="sqd2")
                    for j in range(2):
                        js = slice(128 * j, 128 * (j + 1))
                        STT(h2T[:, j, :], s23b[:, 512 + 128 * j:640 + 128 * j],
                            1.0, tc2[:, js], OP.mult, OP.mult,
                            accum_out=mvS[:, 2 + j:3 + j])
                        STT(sqd2, h2T[:, j, :], 1.0, h2T[:, j, :],
                            OP.mult, OP.mult, accum_out=mvQ[:, 2 + j:3 + j])
                if 4 <= s <= R + 3:
                    h3 = work.tile([128, 32], f16, tag="h3")
                    nc.gpsimd.tensor_tensor(h3, s23b[:, 1088:1120],
                                            tc3, OP.mult)
                    h3tp = smp.tile([32, 128], f16, tag="h3tp")
                    nc.tensor.transpose(h3tp, h3, ident_f)
                    mvS1, mvQ1 = get_mv(s + 1)
                    h3cur = h3q[s % 3]
                    STT(h3cur[0:32, :], h3tp, 0.0, ones32, OP.add, OP.mult,
                        accum_out=mvS1[0:32, 4:5])
                    sqd3 = work.tile([32, 128], bf16, tag="sqd3")
                    STT(sqd3, h3cur[0:32, :], 1.0, h3cur[0:32, :],
                        OP.mult, OP.mult, accum_out=mvQ1[0:32, 4:5])

                # head output for t=s-7: (sigma(d), sigma(-d))
                if 7 <= s:
                    t_out = s - 7
                    nc.gpsimd.tensor_copy(
                        out_sb[:, 2 * t_out:2 * t_out + 2],
                        s23b[:, 1152:1154])

            nc.sync.dma_start(y, out_sb)

    nc.compile()
    return nc


def _perm_and_double(W, b, H):
    """Reorder gate rows (i,f,g,o) -> (i,f,o,g) and double the g section."""
    idx = np.arange(4 * H)
    i, f, g, o = np.split(idx, 4)
    perm = np.concatenate([i, f, o, g])
    Wp = np.asarray(W, np.float64)[perm].copy()
    bp = np.asarray(b, np.float64)[perm].copy()
    Wp[3 * H:] *= 2.0
    bp[3 * H:] *= 2.0
    return Wp, bp


def _prep_host(inputs):
    import ml_dtypes
    bf = ml_dtypes.bfloat16
    f8 = ml_dtypes.float8_e4m3fn

    def to8(a):
        return np.asarray(a, np.float32).astype(f8)

    def hi_lo(v):
        hi = to8(v)
        lo = (np.asarray(v, np.float32) - hi.astype(np.float32)).astype(f8)
        return hi, lo

    W1, b1g = _perm_and_double(inputs["Wih1"],
                               inputs["bih1"] + inputs["bhh1"], H1)
    Wh1, _ = _perm_and_double(inputs["Whh1"], np.zeros(4 * H1), H1)
    W2, b2g = _perm_and_double(inputs["Wih2"],
                               inputs["bih2"] + inputs["bhh2"], H2)
    Wh2, _ = _perm_and_double(inputs["Whh2"], np.zeros(4 * H2), H2)
    W3, b3g = _perm_and_double(inputs["Wih3"],
                               inputs["bih3"] + inputs["bhh3"], H3)
    Wh3, _ = _perm_and_double(inputs["Whh3"], np.zeros(4 * H3), H3)

    # L1 x-side stationary [65, 2, 8, 128]: feature planes of 64 + bias row
    w1x = np.zeros((65, 2, 8, 128), np.float32)
    for c in range(8):
        blk = W1[128 * c:128 * (c + 1)]        # [128 gates, 128 feats]
        w1x[0:64, 0, c, :] = blk[:, 0:64].T
        w1x[0:64, 1, c, :] = blk[:, 64:128].T
    bhi, blo = hi_lo(b1g)
    w1x8 = w1x.astype(f8)
    for c in range(8):
        w1x8[64, 0, c, :] = bhi[128 * c:128 * (c + 1)]
        w1x8[64, 1, c, :] = blo[128 * c:128 * (c + 1)]

    def dr_w(W):                                # [4H, 256] -> [128, 2, 8, 128]
        out = np.zeros((128, 2, 8, 128), np.float32)
        for c in range(8):
            blk = W[128 * c:128 * (c + 1)]      # [128 gates, 256 feats]
            out[:, 0, c, :] = blk[:, 0:128].T
            out[:, 1, c, :] = blk[:, 128:256].T
        return out.astype(f8)

    wh1 = dr_w(Wh1)
    w2z = dr_w(W2)
    wh2 = dr_w(Wh2)

    b2hi, b2lo = hi_lo(b2g)
    b2l = np.zeros((1, 2, 8, 128), np.float32).astype(f8)
    for c in range(8):
        b2l[0, 0, c, :] = b2hi[128 * c:128 * (c + 1)]
        b2l[0, 1, c, :] = b2lo[128 * c:128 * (c + 1)]

    # L3: moving weights w3t [128, 2, 128]: [k-feat, plane, gate]
    w3t = np.zeros((128, 2, 128), np.float32)
    w3t[:, 0, :] = W3[:, 0:128].T
    w3t[:, 1, :] = W3[:, 128:256].T
    w3t = w3t.astype(f8)
    wh3a = np.zeros((33, 128), np.float32)
    wh3a[0:32, :] = Wh3.T
    wh3a[32, :] = b3g
    wh3a = wh3a.astype(bf)

    wlz = np.zeros((33, 2), np.float32)
    wlz[0:32, :] = np.asarray(inputs["Wl"], np.float32).T
    wlz[32, :] = np.asarray(inputs["bl"], np.float32)
    wlz = wlz.astype(bf)

    wdiff = np.asarray(inputs["Wl2"][0] - inputs["Wl2"][1], np.float32)
    dc = float(inputs["bl2"][0] - inputs["bl2"][1])
    wd2 = np.zeros((3, 2), np.float32)
    wd2[0:2, 0] = wdiff
    wd2[0:2, 1] = -wdiff
    wd2[2, 0] = dc
    wd2[2, 1] = -dc
    wd2 = wd2.astype(bf)

    def pack5(g1, g2, g3):
        out = np.zeros((128, 5), np.float32)
        out[:, 0:2] = np.asarray(g1, np.float32).reshape(2, 128).T
        out[:, 2:4] = np.asarray(g2, np.float32).reshape(2, 128).T
        out[0:32, 4] = np.asarray(g3, np.float32)
        return out

    gam = pack5(inputs["g1"], inputs["g2"], inputs["g3"])
    bet = pack5(inputs["b1"], inputs["b2"], inputs["b3"])
    onesd = np.ones((1, 2, 128), np.float32).astype(f8)

    shared = dict(w1x=w1x8, wh1=wh1, w2z=w2z, wh2=wh2, b2l=b2l, w3t=w3t,
                  wh3a=wh3a, wlz=wlz, wd2=wd2, gam=gam, bet=bet, onesd=onesd)

    x = np.asarray(inputs["x"], np.float32)
    in_maps = []
    for cid in range(NCORES):
        xc = x[cid * BL:(cid + 1) * BL, :T_STEPS, :]   # [128, T, 128]
        xa = np.ones((65, 2, T_STEPS, 128), np.float32)
        xt = xc.transpose(2, 1, 0)                     # [feat, T, batch]
        xa[0:64, 0] = xt[0:64]
        xa[0:64, 1] = xt[64:128]
        # row 64: plane0 = plane1 = 1.0 (bias hi+lo rider)
        m = dict(shared)
        m["xdr"] = xa.astype(f8)
        in_maps.append(m)
    return in_maps


def kernel(**inputs):
    from concourse import bass_utils

    key = ("v2", T_STEPS)
    if key not in _CACHE:
        _CACHE[key] = _build()
    nc = _CACHE[key]

    in_maps = _prep_host(inputs)
    res = bass_utils.run_bass_kernel_spmd(nc, in_maps,
                                          core_ids=list(range(NCORES)))
    out = np.empty((B, T_STEPS, 2), np.float32)
    for c in range(NCORES):
        out[c * BL:(c + 1) * BL] = res.results[c]["y"].reshape(BL, T_STEPS, 2)
    return out
